# revision 12
# baseline (speedup 1.0000x reference)
"""AttnDecoderRNN step on 8 Trainium2 NeuronCores.

Sharding: batch-parallel attention+GRU (16 batches/core), vocab-parallel
output projection (4000 logits/core for all 128 batches), AllGather(h_new)
+ AllReduce(sum-exp) on-chip collectives in between.

Self-contained: hardcodes shapes from the problem spec.
"""

import sys

sys.path.insert(0, "/opt/trn_rl_repo")

from contextlib import ExitStack

import numpy as np

import concourse.bass as bass
import concourse.mybir as mybir
from concourse import tile

NCORES = 8


def _split_multi_waits(bir_json: bytes) -> bytes:
    """Legalize Tile-emitted BIR for this walrus build: each instruction may
    carry at most ONE sync wait, so hoist extras into standalone
    EventSemaphore instructions (the raw-bass wait_ge form) just before it."""
    import json as _json

    d = _json.loads(bir_json)
    for fn in d.get("functions", []):
        for blk in fn.get("blocks", fn.get("instruction_blocks", [])):
            insts = blk.get("instructions", [])
            out = []
            for inst in insts:
                si = inst.get("sync_info") or {}
                waits = si.get("on_wait") or []
                if len(waits) > 1:
                    for k, w in enumerate(waits[:-1]):
                        out.append(
                            {
                                "opcode": "EventSemaphore",
                                "name": f"{inst['name']}_hw{k}",
                                "engine": inst["engine"],
                                "ins": [],
                                "outs": [],
                                "sync_info": {"on_update": [], "on_wait": [w]},
                                "debug": inst.get("debug"),
                            }
                        )
                    si["on_wait"] = [waits[-1]]
                out.append(inst)
            blk["instructions"] = out
    return _json.dumps(d).encode()


def _install_birfix():
    import concourse.bass_utils as _bu
    import concourse.bass2jax as _b2j

    if getattr(_bu, "_birfix_installed", False):
        return
    orig = _bu.compile_bir_kernel

    def patched(bir_json, tmpdir, neff_name="file.neff"):
        return orig(_split_multi_waits(bytes(bir_json)), tmpdir, neff_name)

    _bu.compile_bir_kernel = patched
    _b2j.compile_bir_kernel = patched
    _bu._birfix_installed = True


B, T, H, V = 128, 2048, 256, 32000
BL = B // NCORES  # 16 batches per core
VL = V // NCORES  # 4000 vocab rows per core
NT = T // 128  # 16 time tiles
NVC = 8  # vocab chunks per core
VC = VL // NVC  # 500

F32 = mybir.dt.float32
BF16 = mybir.dt.bfloat16
AF = mybir.ActivationFunctionType
OP = mybir.AluOpType
AX = mybir.AxisListType


def build_nc(num_devices: int = NCORES):
    nc = bass.Bass(num_devices=num_devices)

    # ---- DRAM I/O (per-core) ----
    enc = nc.dram_tensor("enc", [T, BL * H], F32, kind="ExternalInput")
    emb16 = nc.dram_tensor("emb16", [BL, H], F32, kind="ExternalInput")
    hid = nc.dram_tensor("hid", [BL, H], F32, kind="ExternalInput")
    web = nc.dram_tensor("web", [128, BL * H], BF16, kind="ExternalInput")
    combWT = nc.dram_tensor("combWT", [2 * H, H], F32, kind="ExternalInput")
    combB = nc.dram_tensor("combB", [128, 2], F32, kind="ExternalInput")
    wihT = nc.dram_tensor("wihT", [H, 3 * H], F32, kind="ExternalInput")
    whhT = nc.dram_tensor("whhT", [H, 3 * H], F32, kind="ExternalInput")
    brz = nc.dram_tensor("brz", [128, 4], F32, kind="ExternalInput")
    bihn = nc.dram_tensor("bihn", [128, 2], F32, kind="ExternalInput")
    bhhn = nc.dram_tensor("bhhn", [128, 2], F32, kind="ExternalInput")
    owT = nc.dram_tensor("owT", [H, VL], BF16, kind="ExternalInput")
    ob = nc.dram_tensor("ob", [1, VL], BF16, kind="ExternalInput")
    onesrb = nc.dram_tensor("onesrb", [1, 128], BF16, kind="ExternalInput")
    onescol = nc.dram_tensor("onescol", [128, 1], F32, kind="ExternalInput")
    onesrow = nc.dram_tensor("onesrow", [1, 128], F32, kind="ExternalInput")
    ident = nc.dram_tensor("ident", [128, 128], F32, kind="ExternalInput")

    attn_o = nc.dram_tensor("attn_o", [BL, T], F32, kind="ExternalOutput")
    ctx_o = nc.dram_tensor("ctx_o", [BL, H], F32, kind="ExternalOutput")
    hnew_o = nc.dram_tensor("hnew_o", [BL, H], F32, kind="ExternalOutput")
    logp_o = nc.dram_tensor("logp_o", [B, VL], F32, kind="ExternalOutput")

    rg = [list(range(num_devices))]

    with tile.TileContext(nc) as tc, ExitStack() as ctx:
        pool = ctx.enter_context(tc.tile_pool(name="sb", bufs=1))
        epool = ctx.enter_context(tc.tile_pool(name="ep", bufs=4))
        spool = ctx.enter_context(tc.tile_pool(name="sp", bufs=4))
        prodp = ctx.enter_context(tc.tile_pool(name="pr", bufs=2))
        pjp = ctx.enter_context(tc.tile_pool(name="pj", bufs=NT))
        dram = ctx.enter_context(tc.tile_pool(name="dr", bufs=1, space="DRAM"))

        # ---- static SBUF loads ----
        def load(dram_t, shape, tag):
            t = pool.tile(shape, F32, tag=tag)
            nc.sync.dma_start(t[:], dram_t[:])
            return t

        # bounce web through ACT so the spine TT mul carries a single
        # (ACT) wait: the TT ISA struct has one sync-wait slot
        web_dma = pool.tile([128, BL * H], BF16, tag="webd")
        nc.sync.dma_start(web_dma[:], web[:])
        web_sb = pool.tile([128, BL * H], BF16, tag="web")
        nc.scalar.copy(web_sb[:], web_dma[:])
        ones_c = load(onescol, [128, 1], "onec")
        ones_r = load(onesrow, [1, 128], "oner")
        id_sb = load(ident, [128, 128], "ident")
        emb_sb = load(emb16, [BL, H], "emb")
        hid_sb = load(hid, [BL, H], "hid")
        combB_sb = load(combB, [128, 2], "combB")
        brz_sb = load(brz, [128, 4], "brz")
        bihn_sb = load(bihn, [128, 2], "bihn")
        bhhn_sb = load(bhhn, [128, 2], "bhhn")
        ob_sb = pool.tile([1, VL], BF16, tag="ob")
        nc.sync.dma_start(ob_sb[:], ob[:])
        onesrb_sb = pool.tile([1, 128], BF16, tag="onerb")
        nc.sync.dma_start(onesrb_sb[:], onesrb[:])
        cw_sb = []
        for kc in range(4):
            t = pool.tile([128, H], F32, tag=f"cw{kc}")
            nc.sync.dma_start(t[:], combWT[kc * 128 : (kc + 1) * 128, :])
            cw_sb.append(t)
        wih_sb, whh_sb = [], []
        for kc in range(2):
            t = pool.tile([128, 3 * H], F32, tag=f"wih{kc}")
            nc.sync.dma_start(t[:], wihT[kc * 128 : (kc + 1) * 128, :])
            wih_sb.append(t)
            t = pool.tile([128, 3 * H], F32, tag=f"whh{kc}")
            nc.sync.dma_start(t[:], whhT[kc * 128 : (kc + 1) * 128, :])
            whh_sb.append(t)
        ow_sb = []
        for hc in range(2):
            t = pool.tile([128, VL], BF16, tag=f"ow{hc}")
            nc.sync.dma_start(t[:], owT[hc * 128 : (hc + 1) * 128, :])
            ow_sb.append(t)

        hallT = pool.tile([128, 2 * B], BF16, tag="hallT")
        se = pool.tile([B, NVC], F32, tag="se")
        nlse = pool.tile([B, 1], F32, tag="nlse")

        with ExitStack() as psctx:
            psum_a = psctx.enter_context(
                tc.tile_pool(name="pa", bufs=1, space="PSUM")
            )

            # ---- Phase A: attention spine ----
            # E tile jj: [128 t, (b h)]; scores via fused mul+reduce on DVE;
            # exp on ACT; PE accumulates unnormalized ctx = sum_t P[t,b] E[t,b,h]
            ctx_ps = psum_a.tile([BL, BL * H], F32, tag="ctxps")
            p_tiles = []
            for jj in range(NT):
                e_t = epool.tile([128, BL * H], F32, tag="etile")
                nc.sync.dma_start(e_t[:], enc[jj * 128 : (jj + 1) * 128, :])
                e_b = epool.tile([128, BL * H], BF16, tag="ebf")
                nc.scalar.copy(e_b[:], e_t[:])
                prod = prodp.tile([128, BL * H], BF16, tag="prod")
                nc.vector.tensor_mul(prod[:], e_b[:], web_sb[:])
                s_j = spool.tile([128, BL], F32, tag="sj")
                nc.vector.reduce_sum(
                    s_j[:], prod[:].rearrange("p (b h) -> p b h", h=H), axis=AX.X
                )
                p_j = pjp.tile([128, BL], F32, tag="pj")
                nc.scalar.activation(p_j[:], s_j[:], AF.Exp)
                p_jb = pjp.tile([128, BL], BF16, tag="pjb")
                nc.scalar.activation(p_jb[:], s_j[:], AF.Exp)
                p_tiles.append(p_j)
                for nb in range(8):
                    nc.tensor.matmul(
                        ctx_ps[:, nb * 512 : (nb + 1) * 512],
                        p_jb[:],
                        e_b[:, nb * 512 : (nb + 1) * 512],
                        start=(jj == 0),
                        stop=(jj == NT - 1),
                    )

            # ---- softmax normalization ----
            # ctx diag extraction: PSUM -> SBUF -> DRAM, then one diagonal
            # DMA read (DRAM APs are flat, so a (BL*H + H)-stride works)
            ctx_all = pool.tile([BL, BL * H], F32, tag="ctxall")
            nc.scalar.copy(ctx_all[:], ctx_ps[:])
            scr = dram.tile([BL, BL * H], F32, tag="scr")
            nc.sync.dma_start(scr[:], ctx_all[:])
            ctx_raw = pool.tile([BL, H], F32, tag="ctxraw")
            diag_ap = bass.AP(scr[:].tensor, 0, [[BL * H + H, BL], [1, H]])
            nc.sync.dma_start(ctx_raw[:], diag_ap)
            psctx.close()

            psum_t = psctx.enter_context(
                tc.tile_pool(name="pt", bufs=2, space="PSUM")
            )
            psum_g = psctx.enter_context(
                tc.tile_pool(name="pg", bufs=2, space="PSUM")
            )
            psum_a = psctx.enter_context(
                tc.tile_pool(name="pa2", bufs=1, space="PSUM")
            )

            def pe_transpose(dst_ap, src_ap, p, f):
                """dst[f, p] = src[p, f] via PE transpose + ACT copy."""
                t_ps = psum_t.tile([128, 128], F32, tag="tps")
                nc.tensor.transpose(t_ps[:f, :p], src_ap, id_sb[:p, :p])
                nc.scalar.copy(dst_ap, t_ps[:f, :p])

            # rs[t_lo, b] = sum_jj P_jj[t_lo, b]
            rs = pool.tile([128, BL], F32, tag="rs")
            nc.vector.tensor_add(rs[:], p_tiles[0][:], p_tiles[1][:])
            for jj in range(2, NT):
                nc.vector.tensor_add(rs[:], rs[:], p_tiles[jj][:])
            # d_col [16,1], d_row [1,16], then broadcast of 1/d to [128,16]
            sm = psum_a.tile([128, BL], F32, tag="sm")
            nc.tensor.matmul(sm[:BL, :1], rs[:], ones_c[:], start=True, stop=True)
            rec_c = pool.tile([BL, 1], F32, tag="recc")
            nc.vector.reciprocal(rec_c[:], sm[:BL, :1])
            sm2 = psum_a.tile([128, BL], F32, tag="sm")
            nc.tensor.matmul(sm2[:1, :BL], ones_c[:], rs[:], start=True, stop=True)
            rec_r = pool.tile([1, BL], F32, tag="recr")
            nc.vector.reciprocal(rec_r[:], sm2[:1, :BL])
            sm3 = psum_a.tile([128, BL], F32, tag="sm")
            nc.tensor.matmul(sm3[:, :BL], ones_r[:], rec_r[:], start=True, stop=True)
            rec_all = pool.tile([128, BL], F32, tag="recall")
            nc.scalar.copy(rec_all[:], sm3[:, :BL])

            # ctx scaled -> output
            ctx_sb = pool.tile([BL, H], F32, tag="ctxsb")
            nc.vector.tensor_scalar_mul(ctx_sb[:], ctx_raw[:], rec_c[:])
            nc.sync.dma_start(ctx_o[:], ctx_sb[:])

            # attn weights: w_jj = P_jj * rec_all ; W_all [128 t_lo, (jj b)]
            w_all = pool.tile([128, NT * BL], F32, tag="wall")
            for jj in range(NT):
                nc.vector.tensor_mul(
                    w_all[:, jj * BL : (jj + 1) * BL], p_tiles[jj][:], rec_all[:]
                )
            # transpose -> [(jj b), t_lo]; rows are t-contiguous in DRAM
            attn_v = attn_o.rearrange("b (j t) -> j b t", j=NT)
            for half in range(2):
                t_sb = pool.tile([128, 128], F32, tag=f"wt{half}")
                pe_transpose(
                    t_sb[:], w_all[:, half * 128 : (half + 1) * 128], 128, 128
                )
                nc.sync.dma_start(attn_v[half * 8 : (half + 1) * 8], t_sb[:])

            # ---- combine + GRU (all h-major / transposed) ----
            # xT [512, 16] = [embT; ctxT] packed as 4 chunks of 16 cols
            xT = pool.tile([128, 4 * BL], F32, tag="xT")
            for src, base in ((emb_sb, 0), (ctx_sb, 2)):
                for hc in range(2):
                    pe_transpose(
                        xT[:, (base + hc) * BL : (base + hc + 1) * BL],
                        src[:, hc * 128 : (hc + 1) * 128],
                        BL,
                        128,
                    )

            # x2T = relu(combW @ x + comb_b)  [2 chunks of 128 h]
            x2T = pool.tile([128, 2 * BL], F32, tag="x2T")
            for hc in range(2):
                g_ps = psum_g.tile([128, BL], F32, tag="gps")
                for kc in range(4):
                    nc.tensor.matmul(
                        g_ps[:],
                        cw_sb[kc][:, hc * 128 : (hc + 1) * 128],
                        xT[:, kc * BL : (kc + 1) * BL],
                        start=(kc == 0),
                        stop=(kc == 3),
                    )
                nc.scalar.activation(
                    x2T[:, hc * BL : (hc + 1) * BL],
                    g_ps[:],
                    AF.Relu,
                    bias=combB_sb[:, hc : hc + 1],
                )

            # hT [256, 16]
            hT = pool.tile([128, 2 * BL], F32, tag="hT")
            for hc in range(2):
                pe_transpose(
                    hT[:, hc * BL : (hc + 1) * BL],
                    hid_sb[:, hc * 128 : (hc + 1) * 128],
                    BL,
                    128,
                )

            # gates r, z: sigmoid(Wih_g @ x2 + Whh_g @ h + (bih+bhh)_g)
            rz = pool.tile([128, 4 * BL], F32, tag="rz")  # r0 r1 z0 z1
            for gc in range(4):
                g_ps = psum_g.tile([128, BL], F32, tag="gps")
                for kc in range(2):
                    nc.tensor.matmul(
                        g_ps[:],
                        wih_sb[kc][:, gc * 128 : (gc + 1) * 128],
                        x2T[:, kc * BL : (kc + 1) * BL],
                        start=(kc == 0),
                        stop=False,
                    )
                for kc in range(2):
                    nc.tensor.matmul(
                        g_ps[:],
                        whh_sb[kc][:, gc * 128 : (gc + 1) * 128],
                        hT[:, kc * BL : (kc + 1) * BL],
                        start=False,
                        stop=(kc == 1),
                    )
                nc.scalar.activation(
                    rz[:, gc * BL : (gc + 1) * BL],
                    g_ps[:],
                    AF.Sigmoid,
                    bias=brz_sb[:, gc : gc + 1],
                )

            # n = tanh(gx_n + bih_n + r * (gh_n + bhh_n)); h' = n + z*(h-n)
            hnT = pool.tile([128, 2 * BL], F32, tag="hnT")
            for i in range(2):
                gc = 4 + i
                gx_ps = psum_g.tile([128, BL], F32, tag="gps")
                for kc in range(2):
                    nc.tensor.matmul(
                        gx_ps[:],
                        wih_sb[kc][:, gc * 128 : (gc + 1) * 128],
                        x2T[:, kc * BL : (kc + 1) * BL],
                        start=(kc == 0),
                        stop=(kc == 1),
                    )
                gh_ps = psum_g.tile([128, BL], F32, tag="gps")
                for kc in range(2):
                    nc.tensor.matmul(
                        gh_ps[:],
                        whh_sb[kc][:, gc * 128 : (gc + 1) * 128],
                        hT[:, kc * BL : (kc + 1) * BL],
                        start=(kc == 0),
                        stop=(kc == 1),
                    )
                ghn = spool.tile([128, BL], F32, tag="ghn")
                nc.scalar.activation(
                    ghn[:], gh_ps[:], AF.Identity, bias=bhhn_sb[:, i : i + 1]
                )
                t1 = spool.tile([128, BL], F32, tag="t1")
                nc.vector.tensor_mul(t1[:], rz[:, i * BL : (i + 1) * BL], ghn[:])
                t2 = spool.tile([128, BL], F32, tag="t2")
                nc.vector.tensor_add(t2[:], gx_ps[:], t1[:])
                nT = spool.tile([128, BL], F32, tag="nT")
                nc.scalar.activation(nT[:], t2[:], AF.Tanh, bias=bihn_sb[:, i : i + 1])
                d1 = spool.tile([128, BL], F32, tag="d1")
                nc.vector.tensor_sub(d1[:], hT[:, i * BL : (i + 1) * BL], nT[:])
                d2 = spool.tile([128, BL], F32, tag="d2")
                nc.vector.tensor_mul(d2[:], rz[:, (2 + i) * BL : (3 + i) * BL], d1[:])
                nc.vector.tensor_add(hnT[:, i * BL : (i + 1) * BL], nT[:], d2[:])

            # h_new output (b-major) + collective AllGather
            hn_sb = pool.tile([BL, H], F32, tag="hnsb")
            for hc in range(2):
                pe_transpose(
                    hn_sb[:, hc * 128 : (hc + 1) * 128],
                    hnT[:, hc * BL : (hc + 1) * BL],
                    128,
                    BL,
                )
            nc.sync.dma_start(hnew_o[:], hn_sb[:])

            cc_in = dram.tile([BL, H], F32, tag="ccin")
            cc_out = dram.tile([B, H], F32, tag="ccout")
            nc.gpsimd.dma_start(cc_in[:], hn_sb[:])
            nc.gpsimd.collective_compute(
                "AllGather",
                OP.bypass,
                replica_groups=rg,
                ins=[cc_in.opt()],
                outs=[cc_out.opt()],
            )
            h_all = pool.tile([B, H], F32, tag="hall")
            nc.sync.dma_start(h_all[:], cc_out[:])
            for hc in range(2):
                pe_transpose(
                    hallT[:, hc * B : (hc + 1) * B],
                    h_all[:, hc * 128 : (hc + 1) * 128],
                    128,
                    128,
                )

        # ---- logits (all 128 b x local 4000 v) ----
        with tc.tile_pool(name="plg", bufs=NVC, space="PSUM") as psum_lg:
            lg_ps = []
            for vc in range(NVC):
                ps = psum_lg.tile([B, VC], F32, tag="lgps")
                for hc in range(2):
                    nc.tensor.matmul(
                        ps[:],
                        hallT[:, hc * B : (hc + 1) * B],
                        ow_sb[hc][:, vc * VC : (vc + 1) * VC],
                        start=(hc == 0),
                        stop=False,
                    )
                nc.tensor.matmul(
                    ps[:],
                    onesrb_sb[:],
                    ob_sb[:, vc * VC : (vc + 1) * VC],
                    start=False,
                    stop=True,
                )
                lg_ps.append(ps)
                pexp = prodp.tile([B, VC], F32, tag="pexp")
                nc.scalar.activation(
                    pexp[:], ps[:], AF.Exp, accum_out=se[:, vc : vc + 1]
                )

            se_t = pool.tile([B, 1], F32, tag="set")
            nc.vector.reduce_sum(se_t[:], se[:], axis=AX.X)
            cc2_in = dram.tile([B, 1], F32, tag="cc2in")
            cc2_out = dram.tile([B, 1], F32, tag="cc2out")
            nc.gpsimd.dma_start(cc2_in[:], se_t[:])
            nc.gpsimd.collective_compute(
                "AllReduce",
                OP.add,
                replica_groups=rg,
                ins=[cc2_in.opt()],
                outs=[cc2_out.opt()],
            )
            se_g = pool.tile([B, 1], F32, tag="seg")
            nc.sync.dma_start(se_g[:], cc2_out[:])
            lse = pool.tile([B, 1], F32, tag="lse")
            nc.scalar.activation(lse[:], se_g[:], AF.Ln)
            nc.vector.tensor_scalar_mul(nlse[:], lse[:], -1.0)

            for vc in range(NVC):
                lp = prodp.tile([B, VC], F32, tag="lp")
                nc.scalar.activation(lp[:], lg_ps[vc][:], AF.Identity, bias=nlse[:])
                nc.sync.dma_start(logp_o[:, vc * VC : (vc + 1) * VC], lp[:])

    nc.finalize()
    return nc


def shard_inputs(
    tokens, hidden, encoder_outputs, emb_table, attn_W, attn_b,
    comb_W, comb_b, gru_Wih, gru_Whh, gru_bih, gru_bhh, out_W, out_b,
):
    import ml_dtypes

    f = np.float32
    bf16 = ml_dtypes.bfloat16
    tokens = np.asarray(tokens).astype(np.int64)
    hidden = np.asarray(hidden, f)
    enc = np.asarray(encoder_outputs, f)
    emb_table = np.asarray(emb_table, f)
    web = np.tile(np.asarray(attn_W, f)[:, H:].astype(bf16), (128, BL))
    combWT = np.ascontiguousarray(np.asarray(comb_W, f).T)
    combB = np.ascontiguousarray(np.asarray(comb_b, f).reshape(2, 128).T)
    wihT = np.ascontiguousarray(np.asarray(gru_Wih, f).T)
    whhT = np.ascontiguousarray(np.asarray(gru_Whh, f).T)
    bih = np.asarray(gru_bih, f)
    bhh = np.asarray(gru_bhh, f)
    brz = np.ascontiguousarray((bih + bhh)[: 2 * H].reshape(4, 128).T)
    bihn = np.ascontiguousarray(bih[2 * H :].reshape(2, 128).T)
    bhhn = np.ascontiguousarray(bhh[2 * H :].reshape(2, 128).T)
    out_W = np.asarray(out_W, f)
    out_b = np.asarray(out_b, f)
    shared = dict(
        web=web, combWT=combWT, combB=combB, wihT=wihT, whhT=whhT,
        brz=brz, bihn=bihn, bhhn=bhhn,
        onescol=np.ones((128, 1), f), onesrow=np.ones((1, 128), f),
        onesrb=np.ones((1, 128), bf16),
        ident=np.eye(128, dtype=f),
    )
    in_maps = []
    for c in range(NCORES):
        b0, v0 = c * BL, c * VL
        m = dict(shared)
        m["enc"] = np.ascontiguousarray(
            enc[:, b0 : b0 + BL, :].reshape(T, BL * H)
        )
        m["emb16"] = np.ascontiguousarray(emb_table[tokens[b0 : b0 + BL, 0]])
        m["hid"] = np.ascontiguousarray(hidden[0, b0 : b0 + BL, :])
        m["owT"] = np.ascontiguousarray(out_W[v0 : v0 + VL].T).astype(bf16)
        m["ob"] = np.ascontiguousarray(out_b[None, v0 : v0 + VL]).astype(bf16)
        in_maps.append(m)
    return in_maps


_NC_CACHE = {}


def _get_nc():
    if "nc" not in _NC_CACHE:
        _NC_CACHE["nc"] = build_nc()
    return _NC_CACHE["nc"]


def assemble_outputs(results):
    logp = np.concatenate([r["logp_o"] for r in results], axis=1)[None]
    hnew = np.concatenate([r["hnew_o"] for r in results], axis=0)[None]
    attn = np.concatenate([r["attn_o"] for r in results], axis=0)[..., None]
    ctxo = np.concatenate([r["ctx_o"] for r in results], axis=0)[None]
    return logp, hnew, attn, ctxo


def kernel(**inputs):
    _install_birfix()
    from concourse.bass_utils import run_bass_kernel_spmd

    nc = _get_nc()
    in_maps = shard_inputs(**inputs)
    res = run_bass_kernel_spmd(nc, in_maps, list(range(NCORES)))
    return assemble_outputs(res.results)


# revision 16
# speedup vs baseline: 1.1511x; 1.1511x over previous
"""AttnDecoderRNN step on 8 Trainium2 NeuronCores.

Sharding: batch-parallel attention+GRU (16 batches/core), vocab-parallel
output projection (4000 logits/core for all 128 batches), AllGather(h_new)
+ AllReduce(sum-exp) on-chip collectives in between.

Self-contained: hardcodes shapes from the problem spec.
"""

import sys

sys.path.insert(0, "/opt/trn_rl_repo")

from contextlib import ExitStack

import numpy as np

import concourse.bass as bass
import concourse.mybir as mybir
from concourse import tile

NCORES = 8


def _split_multi_waits(bir_json: bytes) -> bytes:
    """Legalize Tile-emitted BIR for this walrus build: each instruction may
    carry at most ONE sync wait, so hoist extras into standalone
    EventSemaphore instructions (the raw-bass wait_ge form) just before it."""
    import json as _json

    d = _json.loads(bir_json)
    for fn in d.get("functions", []):
        for blk in fn.get("blocks", fn.get("instruction_blocks", [])):
            insts = blk.get("instructions", [])
            out = []
            for inst in insts:
                si = inst.get("sync_info") or {}
                waits = si.get("on_wait") or []
                if len(waits) > 1:
                    for k, w in enumerate(waits[:-1]):
                        out.append(
                            {
                                "opcode": "EventSemaphore",
                                "name": f"{inst['name']}_hw{k}",
                                "engine": inst["engine"],
                                "ins": [],
                                "outs": [],
                                "sync_info": {"on_update": [], "on_wait": [w]},
                                "debug": inst.get("debug"),
                            }
                        )
                    si["on_wait"] = [waits[-1]]
                out.append(inst)
            blk["instructions"] = out
    return _json.dumps(d).encode()


def _install_birfix():
    import concourse.bass_utils as _bu
    import concourse.bass2jax as _b2j

    if getattr(_bu, "_birfix_installed", False):
        return
    orig = _bu.compile_bir_kernel

    def patched(bir_json, tmpdir, neff_name="file.neff"):
        return orig(_split_multi_waits(bytes(bir_json)), tmpdir, neff_name)

    _bu.compile_bir_kernel = patched
    _b2j.compile_bir_kernel = patched
    _bu._birfix_installed = True


B, T, H, V = 128, 2048, 256, 32000
BL = B // NCORES  # 16 batches per core
VL = V // NCORES  # 4000 vocab rows per core
NT = T // 128  # 16 time tiles
NVC = 8  # vocab chunks per core
VC = VL // NVC  # 500

F32 = mybir.dt.float32
BF16 = mybir.dt.bfloat16
AF = mybir.ActivationFunctionType
OP = mybir.AluOpType
AX = mybir.AxisListType


def build_nc(num_devices: int = NCORES):
    nc = bass.Bass(num_devices=num_devices)

    # ---- DRAM I/O (per-core) ----
    enc = nc.dram_tensor("enc", [T, BL * H], F32, kind="ExternalInput")
    emb16 = nc.dram_tensor("emb16", [BL, H], F32, kind="ExternalInput")
    hid = nc.dram_tensor("hid", [BL, H], F32, kind="ExternalInput")
    web = nc.dram_tensor("web", [128, BL * H], BF16, kind="ExternalInput")
    combWT = nc.dram_tensor("combWT", [2 * H, H], F32, kind="ExternalInput")
    combB = nc.dram_tensor("combB", [128, 2], F32, kind="ExternalInput")
    wihT = nc.dram_tensor("wihT", [H, 3 * H], F32, kind="ExternalInput")
    whhT = nc.dram_tensor("whhT", [H, 3 * H], F32, kind="ExternalInput")
    brz = nc.dram_tensor("brz", [128, 4], F32, kind="ExternalInput")
    bihn = nc.dram_tensor("bihn", [128, 2], F32, kind="ExternalInput")
    bhhn = nc.dram_tensor("bhhn", [128, 2], F32, kind="ExternalInput")
    owT = nc.dram_tensor("owT", [H, VL], BF16, kind="ExternalInput")
    ob = nc.dram_tensor("ob", [1, VL], BF16, kind="ExternalInput")
    onesrb = nc.dram_tensor("onesrb", [1, 128], BF16, kind="ExternalInput")
    onescol = nc.dram_tensor("onescol", [128, 1], F32, kind="ExternalInput")
    onesrow = nc.dram_tensor("onesrow", [1, 128], F32, kind="ExternalInput")
    ident = nc.dram_tensor("ident", [128, 128], F32, kind="ExternalInput")

    attn_o = nc.dram_tensor("attn_o", [BL, T], F32, kind="ExternalOutput")
    ctx_o = nc.dram_tensor("ctx_o", [BL, H], F32, kind="ExternalOutput")
    hnew_o = nc.dram_tensor("hnew_o", [BL, H], F32, kind="ExternalOutput")
    logp_o = nc.dram_tensor("logp_o", [B, VL], F32, kind="ExternalOutput")

    rg = [list(range(num_devices))]

    with tile.TileContext(nc) as tc, ExitStack() as ctx:
        pool = ctx.enter_context(tc.tile_pool(name="sb", bufs=1))
        epool = ctx.enter_context(tc.tile_pool(name="ep", bufs=4))
        spool = ctx.enter_context(tc.tile_pool(name="sp", bufs=4))
        prodp = ctx.enter_context(tc.tile_pool(name="pr", bufs=2))
        pjp = ctx.enter_context(tc.tile_pool(name="pj", bufs=NT))
        dram = ctx.enter_context(tc.tile_pool(name="dr", bufs=1, space="DRAM"))

        # ---- static SBUF loads ----
        def load(dram_t, shape, tag):
            t = pool.tile(shape, F32, tag=tag)
            nc.sync.dma_start(t[:], dram_t[:])
            return t

        # bounce web through ACT so the spine TT mul carries a single
        # (ACT) wait: the TT ISA struct has one sync-wait slot
        web_dma = pool.tile([128, BL * H], BF16, tag="webd")
        nc.sync.dma_start(web_dma[:], web[:])
        web_sb = pool.tile([128, BL * H], BF16, tag="web")
        nc.scalar.copy(web_sb[:], web_dma[:])
        ones_c = load(onescol, [128, 1], "onec")
        ones_r = load(onesrow, [1, 128], "oner")
        id_sb = load(ident, [128, 128], "ident")
        emb_sb = load(emb16, [BL, H], "emb")
        hid_sb = load(hid, [BL, H], "hid")
        combB_sb = load(combB, [128, 2], "combB")
        brz_sb = load(brz, [128, 4], "brz")
        bihn_sb = load(bihn, [128, 2], "bihn")
        bhhn_sb = load(bhhn, [128, 2], "bhhn")
        ob_sb = pool.tile([1, VL], BF16, tag="ob")
        nc.sync.dma_start(ob_sb[:], ob[:])
        onesrb_sb = pool.tile([1, 128], BF16, tag="onerb")
        nc.sync.dma_start(onesrb_sb[:], onesrb[:])

        hallT = pool.tile([128, 2 * B], BF16, tag="hallT")
        se = pool.tile([B, NVC], F32, tag="se")
        nlse = pool.tile([B, 1], F32, tag="nlse")

        with ExitStack() as psctx:
            psum_a = psctx.enter_context(
                tc.tile_pool(name="pa", bufs=1, space="PSUM")
            )

            # ---- Phase A: attention spine ----
            # E tile jj: [128 t, (b h)]; scores via fused mul+reduce on DVE;
            # exp on ACT; PE accumulates unnormalized ctx = sum_t P[t,b] E[t,b,h]
            ctx_ps = psum_a.tile([BL, BL * H], F32, tag="ctxps")
            p_tiles = []
            for jj in range(NT):
                e_t = epool.tile([128, BL * H], F32, tag="etile")
                nc.sync.dma_start(e_t[:], enc[jj * 128 : (jj + 1) * 128, :])
                e_b = epool.tile([128, BL * H], BF16, tag="ebf")
                nc.scalar.copy(e_b[:], e_t[:])
                prod = prodp.tile([128, BL * H], BF16, tag="prod")
                nc.vector.tensor_mul(prod[:], e_b[:], web_sb[:])
                p3 = prod[:].rearrange("p (b h) -> p b h", h=H)
                nc.vector.tensor_add(p3[:, :, :128], p3[:, :, :128], p3[:, :, 128:])
                nc.vector.tensor_add(p3[:, :, :64], p3[:, :, :64], p3[:, :, 64:128])
                s_j = spool.tile([128, BL], F32, tag="sj")
                nc.vector.reduce_sum(s_j[:], p3[:, :, :64], axis=AX.X)
                p_j = pjp.tile([128, BL], F32, tag="pj")
                nc.scalar.activation(p_j[:], s_j[:], AF.Exp)
                p_jb = pjp.tile([128, BL], BF16, tag="pjb")
                nc.scalar.activation(p_jb[:], s_j[:], AF.Exp)
                p_tiles.append(p_j)
                for nb in range(8):
                    nc.tensor.matmul(
                        ctx_ps[:, nb * 512 : (nb + 1) * 512],
                        p_jb[:],
                        e_b[:, nb * 512 : (nb + 1) * 512],
                        start=(jj == 0),
                        stop=(jj == NT - 1),
                    )

            # deferred weight loads (after E tiles so the spine owns the
            # DMA queues at startup)
            cw_sb = []
            for kc in range(4):
                t = pool.tile([128, H], F32, tag=f"cw{kc}")
                nc.sync.dma_start(t[:], combWT[kc * 128 : (kc + 1) * 128, :])
                cw_sb.append(t)
            wih_sb, whh_sb = [], []
            for kc in range(2):
                t = pool.tile([128, 3 * H], F32, tag=f"wih{kc}")
                nc.sync.dma_start(t[:], wihT[kc * 128 : (kc + 1) * 128, :])
                wih_sb.append(t)
                t = pool.tile([128, 3 * H], F32, tag=f"whh{kc}")
                nc.sync.dma_start(t[:], whhT[kc * 128 : (kc + 1) * 128, :])
                whh_sb.append(t)
            ow_sb = []
            for hc in range(2):
                t = pool.tile([128, VL], BF16, tag=f"ow{hc}")
                nc.sync.dma_start(t[:], owT[hc * 128 : (hc + 1) * 128, :])
                ow_sb.append(t)

            # ---- softmax normalization ----
            # ctx diag extraction: PSUM -> SBUF -> DRAM, then one diagonal
            # DMA read (DRAM APs are flat, so a (BL*H + H)-stride works)
            ctx_all = pool.tile([BL, BL * H], F32, tag="ctxall")
            nc.scalar.copy(ctx_all[:], ctx_ps[:])
            scr = dram.tile([BL, BL * H], F32, tag="scr")
            nc.sync.dma_start(scr[:], ctx_all[:])
            ctx_raw = pool.tile([BL, H], F32, tag="ctxraw")
            diag_ap = bass.AP(scr[:].tensor, 0, [[BL * H + H, BL], [1, H]])
            nc.sync.dma_start(ctx_raw[:], diag_ap)
            psctx.close()

            psum_t = psctx.enter_context(
                tc.tile_pool(name="pt", bufs=2, space="PSUM")
            )
            psum_g = psctx.enter_context(
                tc.tile_pool(name="pg", bufs=2, space="PSUM")
            )
            psum_a = psctx.enter_context(
                tc.tile_pool(name="pa2", bufs=1, space="PSUM")
            )

            def pe_transpose(dst_ap, src_ap, p, f):
                """dst[f, p] = src[p, f] via PE transpose + ACT copy."""
                t_ps = psum_t.tile([128, 128], F32, tag="tps")
                nc.tensor.transpose(t_ps[:f, :p], src_ap, id_sb[:p, :p])
                nc.scalar.copy(dst_ap, t_ps[:f, :p])

            # rs[t_lo, b] = sum_jj P_jj[t_lo, b]
            rs = pool.tile([128, BL], F32, tag="rs")
            nc.vector.tensor_add(rs[:], p_tiles[0][:], p_tiles[1][:])
            for jj in range(2, NT):
                nc.vector.tensor_add(rs[:], rs[:], p_tiles[jj][:])
            # d_col [16,1], d_row [1,16], then broadcast of 1/d to [128,16]
            sm = psum_a.tile([128, BL], F32, tag="sm")
            nc.tensor.matmul(sm[:BL, :1], rs[:], ones_c[:], start=True, stop=True)
            rec_c = pool.tile([BL, 1], F32, tag="recc")
            nc.vector.reciprocal(rec_c[:], sm[:BL, :1])
            # ctx scaled (stores + attn path deferred past the AllGather)
            ctx_sb = pool.tile([BL, H], F32, tag="ctxsb")
            nc.vector.tensor_scalar_mul(ctx_sb[:], ctx_raw[:], rec_c[:])

            # ---- combine + GRU (all h-major / transposed) ----
            # xT [512, 16] = [embT; ctxT] packed as 4 chunks of 16 cols
            xT = pool.tile([128, 4 * BL], F32, tag="xT")
            for src, base in ((emb_sb, 0), (ctx_sb, 2)):
                for hc in range(2):
                    pe_transpose(
                        xT[:, (base + hc) * BL : (base + hc + 1) * BL],
                        src[:, hc * 128 : (hc + 1) * 128],
                        BL,
                        128,
                    )

            # x2T = relu(combW @ x + comb_b)  [2 chunks of 128 h]
            x2T = pool.tile([128, 2 * BL], F32, tag="x2T")
            for hc in range(2):
                g_ps = psum_g.tile([128, BL], F32, tag="gps")
                for kc in range(4):
                    nc.tensor.matmul(
                        g_ps[:],
                        cw_sb[kc][:, hc * 128 : (hc + 1) * 128],
                        xT[:, kc * BL : (kc + 1) * BL],
                        start=(kc == 0),
                        stop=(kc == 3),
                    )
                nc.scalar.activation(
                    x2T[:, hc * BL : (hc + 1) * BL],
                    g_ps[:],
                    AF.Relu,
                    bias=combB_sb[:, hc : hc + 1],
                )

            # hT [256, 16]
            hT = pool.tile([128, 2 * BL], F32, tag="hT")
            for hc in range(2):
                pe_transpose(
                    hT[:, hc * BL : (hc + 1) * BL],
                    hid_sb[:, hc * 128 : (hc + 1) * 128],
                    BL,
                    128,
                )

            # gates r, z: sigmoid(Wih_g @ x2 + Whh_g @ h + (bih+bhh)_g)
            rz = pool.tile([128, 4 * BL], F32, tag="rz")  # r0 r1 z0 z1
            for gc in range(4):
                g_ps = psum_g.tile([128, BL], F32, tag="gps")
                for kc in range(2):
                    nc.tensor.matmul(
                        g_ps[:],
                        wih_sb[kc][:, gc * 128 : (gc + 1) * 128],
                        x2T[:, kc * BL : (kc + 1) * BL],
                        start=(kc == 0),
                        stop=False,
                    )
                for kc in range(2):
                    nc.tensor.matmul(
                        g_ps[:],
                        whh_sb[kc][:, gc * 128 : (gc + 1) * 128],
                        hT[:, kc * BL : (kc + 1) * BL],
                        start=False,
                        stop=(kc == 1),
                    )
                nc.scalar.activation(
                    rz[:, gc * BL : (gc + 1) * BL],
                    g_ps[:],
                    AF.Sigmoid,
                    bias=brz_sb[:, gc : gc + 1],
                )

            # n = tanh(gx_n + bih_n + r * (gh_n + bhh_n)); h' = n + z*(h-n)
            hnT = pool.tile([128, 2 * BL], F32, tag="hnT")
            for i in range(2):
                gc = 4 + i
                gx_ps = psum_g.tile([128, BL], F32, tag="gps")
                for kc in range(2):
                    nc.tensor.matmul(
                        gx_ps[:],
                        wih_sb[kc][:, gc * 128 : (gc + 1) * 128],
                        x2T[:, kc * BL : (kc + 1) * BL],
                        start=(kc == 0),
                        stop=(kc == 1),
                    )
                gh_ps = psum_g.tile([128, BL], F32, tag="gps")
                for kc in range(2):
                    nc.tensor.matmul(
                        gh_ps[:],
                        whh_sb[kc][:, gc * 128 : (gc + 1) * 128],
                        hT[:, kc * BL : (kc + 1) * BL],
                        start=(kc == 0),
                        stop=(kc == 1),
                    )
                ghn = spool.tile([128, BL], F32, tag="ghn")
                nc.scalar.activation(
                    ghn[:], gh_ps[:], AF.Identity, bias=bhhn_sb[:, i : i + 1]
                )
                t1 = spool.tile([128, BL], F32, tag="t1")
                nc.vector.tensor_mul(t1[:], rz[:, i * BL : (i + 1) * BL], ghn[:])
                t2 = spool.tile([128, BL], F32, tag="t2")
                nc.vector.tensor_add(t2[:], gx_ps[:], t1[:])
                nT = spool.tile([128, BL], F32, tag="nT")
                nc.scalar.activation(nT[:], t2[:], AF.Tanh, bias=bihn_sb[:, i : i + 1])
                d1 = spool.tile([128, BL], F32, tag="d1")
                nc.vector.tensor_sub(d1[:], hT[:, i * BL : (i + 1) * BL], nT[:])
                d2 = spool.tile([128, BL], F32, tag="d2")
                nc.vector.tensor_mul(d2[:], rz[:, (2 + i) * BL : (3 + i) * BL], d1[:])
                nc.vector.tensor_add(hnT[:, i * BL : (i + 1) * BL], nT[:], d2[:])

            # h_new output (b-major) + collective AllGather
            hn_sb = pool.tile([BL, H], F32, tag="hnsb")
            for hc in range(2):
                pe_transpose(
                    hn_sb[:, hc * 128 : (hc + 1) * 128],
                    hnT[:, hc * BL : (hc + 1) * BL],
                    128,
                    BL,
                )
            cc_in = dram.tile([BL, H], F32, tag="ccin")
            cc_out = dram.tile([B, H], F32, tag="ccout")
            nc.sync.dma_start(cc_in[:], hn_sb[:])
            nc.gpsimd.collective_compute(
                "AllGather",
                OP.bypass,
                replica_groups=rg,
                ins=[cc_in.opt()],
                outs=[cc_out.opt()],
            )

            # overlapped with the AllGather: output stores + attn_w path
            nc.sync.dma_start(hnew_o[:], hn_sb[:])
            nc.sync.dma_start(ctx_o[:], ctx_sb[:])
            sm2 = psum_a.tile([128, BL], F32, tag="sm")
            nc.tensor.matmul(sm2[:1, :BL], ones_c[:], rs[:], start=True, stop=True)
            rec_r = pool.tile([1, BL], F32, tag="recr")
            nc.vector.reciprocal(rec_r[:], sm2[:1, :BL])
            sm3 = psum_a.tile([128, BL], F32, tag="sm")
            nc.tensor.matmul(sm3[:, :BL], ones_r[:], rec_r[:], start=True, stop=True)
            rec_all = pool.tile([128, BL], F32, tag="recall")
            nc.scalar.copy(rec_all[:], sm3[:, :BL])
            w_all = pool.tile([128, NT * BL], F32, tag="wall")
            for jj in range(NT):
                nc.vector.tensor_mul(
                    w_all[:, jj * BL : (jj + 1) * BL], p_tiles[jj][:], rec_all[:]
                )
            attn_v = attn_o.rearrange("b (j t) -> j b t", j=NT)
            for half in range(2):
                t_sb = pool.tile([128, 128], F32, tag=f"wt{half}")
                pe_transpose(
                    t_sb[:], w_all[:, half * 128 : (half + 1) * 128], 128, 128
                )
                nc.sync.dma_start(attn_v[half * 8 : (half + 1) * 8], t_sb[:])

            h_all = pool.tile([B, H], F32, tag="hall")
            nc.sync.dma_start(h_all[:], cc_out[:])
            for hc in range(2):
                pe_transpose(
                    hallT[:, hc * B : (hc + 1) * B],
                    h_all[:, hc * 128 : (hc + 1) * 128],
                    128,
                    128,
                )

        # ---- logits (all 128 b x local 4000 v) ----
        with tc.tile_pool(name="plg", bufs=NVC, space="PSUM") as psum_lg:
            lg_ps = []
            for vc in range(NVC):
                ps = psum_lg.tile([B, VC], F32, tag="lgps")
                for hc in range(2):
                    nc.tensor.matmul(
                        ps[:],
                        hallT[:, hc * B : (hc + 1) * B],
                        ow_sb[hc][:, vc * VC : (vc + 1) * VC],
                        start=(hc == 0),
                        stop=False,
                    )
                nc.tensor.matmul(
                    ps[:],
                    onesrb_sb[:],
                    ob_sb[:, vc * VC : (vc + 1) * VC],
                    start=False,
                    stop=True,
                )
                lg_ps.append(ps)
                pexp = prodp.tile([B, VC], F32, tag="pexp")
                nc.scalar.activation(
                    pexp[:], ps[:], AF.Exp, accum_out=se[:, vc : vc + 1]
                )

            se_t = pool.tile([B, 1], F32, tag="set")
            nc.vector.reduce_sum(se_t[:], se[:], axis=AX.X)
            cc2_in = dram.tile([B, 1], F32, tag="cc2in")
            cc2_out = dram.tile([B, 1], F32, tag="cc2out")
            nc.sync.dma_start(cc2_in[:], se_t[:])
            nc.gpsimd.collective_compute(
                "AllReduce",
                OP.add,
                replica_groups=rg,
                ins=[cc2_in.opt()],
                outs=[cc2_out.opt()],
            )
            se_g = pool.tile([B, 1], F32, tag="seg")
            nc.sync.dma_start(se_g[:], cc2_out[:])
            lse = pool.tile([B, 1], F32, tag="lse")
            nc.scalar.activation(lse[:], se_g[:], AF.Ln)
            nc.vector.tensor_scalar_mul(nlse[:], lse[:], -1.0)

            for vc in range(NVC):
                lp = prodp.tile([B, VC], F32, tag="lp")
                nc.scalar.activation(lp[:], lg_ps[vc][:], AF.Identity, bias=nlse[:])
                nc.sync.dma_start(logp_o[:, vc * VC : (vc + 1) * VC], lp[:])

    nc.finalize()
    return nc


def shard_inputs(
    tokens, hidden, encoder_outputs, emb_table, attn_W, attn_b,
    comb_W, comb_b, gru_Wih, gru_Whh, gru_bih, gru_bhh, out_W, out_b,
):
    import ml_dtypes

    f = np.float32
    bf16 = ml_dtypes.bfloat16
    tokens = np.asarray(tokens).astype(np.int64)
    hidden = np.asarray(hidden, f)
    enc = np.asarray(encoder_outputs, f)
    emb_table = np.asarray(emb_table, f)
    web = np.tile(np.asarray(attn_W, f)[:, H:].astype(bf16), (128, BL))
    combWT = np.ascontiguousarray(np.asarray(comb_W, f).T)
    combB = np.ascontiguousarray(np.asarray(comb_b, f).reshape(2, 128).T)
    wihT = np.ascontiguousarray(np.asarray(gru_Wih, f).T)
    whhT = np.ascontiguousarray(np.asarray(gru_Whh, f).T)
    bih = np.asarray(gru_bih, f)
    bhh = np.asarray(gru_bhh, f)
    brz = np.ascontiguousarray((bih + bhh)[: 2 * H].reshape(4, 128).T)
    bihn = np.ascontiguousarray(bih[2 * H :].reshape(2, 128).T)
    bhhn = np.ascontiguousarray(bhh[2 * H :].reshape(2, 128).T)
    out_W = np.asarray(out_W, f)
    out_b = np.asarray(out_b, f)
    shared = dict(
        web=web, combWT=combWT, combB=combB, wihT=wihT, whhT=whhT,
        brz=brz, bihn=bihn, bhhn=bhhn,
        onescol=np.ones((128, 1), f), onesrow=np.ones((1, 128), f),
        onesrb=np.ones((1, 128), bf16),
        ident=np.eye(128, dtype=f),
    )
    in_maps = []
    for c in range(NCORES):
        b0, v0 = c * BL, c * VL
        m = dict(shared)
        m["enc"] = np.ascontiguousarray(
            enc[:, b0 : b0 + BL, :].reshape(T, BL * H)
        )
        m["emb16"] = np.ascontiguousarray(emb_table[tokens[b0 : b0 + BL, 0]])
        m["hid"] = np.ascontiguousarray(hidden[0, b0 : b0 + BL, :])
        m["owT"] = np.ascontiguousarray(out_W[v0 : v0 + VL].T).astype(bf16)
        m["ob"] = np.ascontiguousarray(out_b[None, v0 : v0 + VL]).astype(bf16)
        in_maps.append(m)
    return in_maps


_NC_CACHE = {}


def _get_nc():
    if "nc" not in _NC_CACHE:
        _NC_CACHE["nc"] = build_nc()
    return _NC_CACHE["nc"]


def assemble_outputs(results):
    logp = np.concatenate([r["logp_o"] for r in results], axis=1)[None]
    hnew = np.concatenate([r["hnew_o"] for r in results], axis=0)[None]
    attn = np.concatenate([r["attn_o"] for r in results], axis=0)[..., None]
    ctxo = np.concatenate([r["ctx_o"] for r in results], axis=0)[None]
    return logp, hnew, attn, ctxo


def kernel(**inputs):
    _install_birfix()
    from concourse.bass_utils import run_bass_kernel_spmd

    nc = _get_nc()
    in_maps = shard_inputs(**inputs)
    res = run_bass_kernel_spmd(nc, in_maps, list(range(NCORES)))
    return assemble_outputs(res.results)


# revision 18
# speedup vs baseline: 1.1754x; 1.0211x over previous
"""AttnDecoderRNN step on 8 Trainium2 NeuronCores.

Sharding: batch-parallel attention+GRU (16 batches/core), vocab-parallel
output projection (4000 logits/core for all 128 batches), AllGather(h_new)
+ AllReduce(sum-exp) on-chip collectives in between.

Self-contained: hardcodes shapes from the problem spec.
"""

import sys

sys.path.insert(0, "/opt/trn_rl_repo")

from contextlib import ExitStack

import numpy as np

import concourse.bass as bass
import concourse.mybir as mybir
from concourse import tile

NCORES = 8


def _split_multi_waits(bir_json: bytes) -> bytes:
    """Legalize Tile-emitted BIR for this walrus build: each instruction may
    carry at most ONE sync wait, so hoist extras into standalone
    EventSemaphore instructions (the raw-bass wait_ge form) just before it."""
    import json as _json

    d = _json.loads(bir_json)
    for fn in d.get("functions", []):
        for blk in fn.get("blocks", fn.get("instruction_blocks", [])):
            insts = blk.get("instructions", [])
            out = []
            for inst in insts:
                si = inst.get("sync_info") or {}
                waits = si.get("on_wait") or []
                if len(waits) > 1:
                    for k, w in enumerate(waits[:-1]):
                        out.append(
                            {
                                "opcode": "EventSemaphore",
                                "name": f"{inst['name']}_hw{k}",
                                "engine": inst["engine"],
                                "ins": [],
                                "outs": [],
                                "sync_info": {"on_update": [], "on_wait": [w]},
                                "debug": inst.get("debug"),
                            }
                        )
                    si["on_wait"] = [waits[-1]]
                out.append(inst)
            blk["instructions"] = out
    return _json.dumps(d).encode()


def _install_birfix():
    import concourse.bass_utils as _bu
    import concourse.bass2jax as _b2j

    if getattr(_bu, "_birfix_installed", False):
        return
    orig = _bu.compile_bir_kernel

    def patched(bir_json, tmpdir, neff_name="file.neff"):
        return orig(_split_multi_waits(bytes(bir_json)), tmpdir, neff_name)

    _bu.compile_bir_kernel = patched
    _b2j.compile_bir_kernel = patched
    _bu._birfix_installed = True


B, T, H, V = 128, 2048, 256, 32000
BL = B // NCORES  # 16 batches per core
VL = V // NCORES  # 4000 vocab rows per core
NT = T // 128  # 16 time tiles
NVC = 8  # vocab chunks per core
VC = VL // NVC  # 500

F32 = mybir.dt.float32
BF16 = mybir.dt.bfloat16
AF = mybir.ActivationFunctionType
OP = mybir.AluOpType
AX = mybir.AxisListType


def build_nc(num_devices: int = NCORES):
    nc = bass.Bass(num_devices=num_devices)

    # ---- DRAM I/O (per-core) ----
    enc = nc.dram_tensor("enc", [T, BL * H], F32, kind="ExternalInput")
    emb16 = nc.dram_tensor("emb16", [BL, H], F32, kind="ExternalInput")
    hid = nc.dram_tensor("hid", [BL, H], F32, kind="ExternalInput")
    web = nc.dram_tensor("web", [128, BL * H], BF16, kind="ExternalInput")
    combWT = nc.dram_tensor("combWT", [2 * H, H], F32, kind="ExternalInput")
    combB = nc.dram_tensor("combB", [128, 2], F32, kind="ExternalInput")
    wihT = nc.dram_tensor("wihT", [H, 3 * H], F32, kind="ExternalInput")
    whhT = nc.dram_tensor("whhT", [H, 3 * H], F32, kind="ExternalInput")
    brz = nc.dram_tensor("brz", [128, 4], F32, kind="ExternalInput")
    bihn = nc.dram_tensor("bihn", [128, 2], F32, kind="ExternalInput")
    bhhn = nc.dram_tensor("bhhn", [128, 2], F32, kind="ExternalInput")
    owT = nc.dram_tensor("owT", [H, VL], BF16, kind="ExternalInput")
    ob = nc.dram_tensor("ob", [1, VL], BF16, kind="ExternalInput")
    onesrb = nc.dram_tensor("onesrb", [1, 128], BF16, kind="ExternalInput")
    onescol = nc.dram_tensor("onescol", [128, 1], F32, kind="ExternalInput")
    onesrow = nc.dram_tensor("onesrow", [1, 128], F32, kind="ExternalInput")
    ident = nc.dram_tensor("ident", [128, 128], F32, kind="ExternalInput")

    attn_o = nc.dram_tensor("attn_o", [BL, T], F32, kind="ExternalOutput")
    ctx_o = nc.dram_tensor("ctx_o", [BL, H], F32, kind="ExternalOutput")
    hnew_o = nc.dram_tensor("hnew_o", [BL, H], F32, kind="ExternalOutput")
    logp_o = nc.dram_tensor("logp_o", [B, VL], F32, kind="ExternalOutput")

    rg = [list(range(num_devices))]

    with tile.TileContext(nc) as tc, ExitStack() as ctx:
        pool = ctx.enter_context(tc.tile_pool(name="sb", bufs=1))
        epool = ctx.enter_context(tc.tile_pool(name="ep", bufs=4))
        spool = ctx.enter_context(tc.tile_pool(name="sp", bufs=4))
        prodp = ctx.enter_context(tc.tile_pool(name="pr", bufs=2))
        pjp = ctx.enter_context(tc.tile_pool(name="pj", bufs=NT))
        dram = ctx.enter_context(tc.tile_pool(name="dr", bufs=1, space="DRAM"))

        # ---- static SBUF loads ----
        def load(dram_t, shape, tag):
            t = pool.tile(shape, F32, tag=tag)
            nc.sync.dma_start(t[:], dram_t[:])
            return t

        # bounce web through ACT so the spine TT mul carries a single
        # (ACT) wait: the TT ISA struct has one sync-wait slot
        web_dma = pool.tile([128, BL * H], BF16, tag="webd")
        nc.sync.dma_start(web_dma[:], web[:])
        web_sb = pool.tile([128, BL * H], BF16, tag="web")
        nc.scalar.copy(web_sb[:], web_dma[:])
        ones_c = load(onescol, [128, 1], "onec")
        ones_r = load(onesrow, [1, 128], "oner")
        id_sb = load(ident, [128, 128], "ident")
        emb_sb = load(emb16, [BL, H], "emb")
        hid_sb = load(hid, [BL, H], "hid")
        combB_sb = load(combB, [128, 2], "combB")
        brz_sb = load(brz, [128, 4], "brz")
        bihn_sb = load(bihn, [128, 2], "bihn")
        bhhn_sb = load(bhhn, [128, 2], "bhhn")
        ob_sb = pool.tile([1, VL], BF16, tag="ob")
        nc.sync.dma_start(ob_sb[:], ob[:])
        onesrb_sb = pool.tile([1, 128], BF16, tag="onerb")
        nc.sync.dma_start(onesrb_sb[:], onesrb[:])

        hallT = pool.tile([128, 2 * B], BF16, tag="hallT")
        se = pool.tile([B, NVC], F32, tag="se")
        nlse = pool.tile([B, 1], F32, tag="nlse")

        with ExitStack() as psctx:
            psum_a = psctx.enter_context(
                tc.tile_pool(name="pa", bufs=1, space="PSUM")
            )

            # ---- Phase A: attention spine ----
            # E tile jj: [128 t, (b h)]; scores via fused mul+reduce on DVE;
            # exp on ACT; PE accumulates unnormalized ctx = sum_t P[t,b] E[t,b,h]
            ctx_ps = psum_a.tile([BL, BL * H], F32, tag="ctxps")
            p_tiles = []
            for jj in range(NT):
                e_t = epool.tile([128, BL * H], F32, tag="etile")
                s_j = spool.tile([128, BL], F32, tag="sj")
                nchunk = 4 if jj == 0 else 1
                gb = BL // nchunk
                for g in range(nchunk):
                    c0, c1 = g * gb * H, (g + 1) * gb * H
                    nc.sync.dma_start(e_t[:, c0:c1], enc[jj * 128 : (jj + 1) * 128, c0:c1])
                    if g == 0:
                        e_b = epool.tile([128, BL * H], BF16, tag="ebf")
                        prod = prodp.tile([128, BL * H], BF16, tag="prod")
                    nc.scalar.copy(e_b[:, c0:c1], e_t[:, c0:c1])
                    nc.vector.tensor_mul(
                        prod[:, c0:c1], e_b[:, c0:c1], web_sb[:, c0:c1]
                    )
                    p3 = prod[:, c0:c1].rearrange("p (b h) -> p b h", h=H)
                    nc.vector.tensor_add(
                        p3[:, :, :128], p3[:, :, :128], p3[:, :, 128:]
                    )
                    nc.vector.tensor_add(
                        p3[:, :, :64], p3[:, :, :64], p3[:, :, 64:128]
                    )
                    nc.vector.reduce_sum(
                        s_j[:, g * gb : (g + 1) * gb], p3[:, :, :64], axis=AX.X
                    )
                p_j = pjp.tile([128, BL], F32, tag="pj")
                nc.scalar.activation(p_j[:], s_j[:], AF.Exp)
                p_jb = pjp.tile([128, BL], BF16, tag="pjb")
                nc.scalar.activation(p_jb[:], s_j[:], AF.Exp)
                p_tiles.append(p_j)
                if jj == 1:
                    rs = pool.tile([128, BL], F32, tag="rs")
                    nc.vector.tensor_add(rs[:], p_tiles[0][:], p_tiles[1][:])
                elif jj > 1:
                    nc.vector.tensor_add(rs[:], rs[:], p_j[:])
                for nb in range(8):
                    nc.tensor.matmul(
                        ctx_ps[:, nb * 512 : (nb + 1) * 512],
                        p_jb[:],
                        e_b[:, nb * 512 : (nb + 1) * 512],
                        start=(jj == 0),
                        stop=(jj == NT - 1),
                    )

            # deferred weight loads (after E tiles so the spine owns the
            # DMA queues at startup)
            cw_sb = []
            for kc in range(4):
                t = pool.tile([128, H], F32, tag=f"cw{kc}")
                nc.sync.dma_start(t[:], combWT[kc * 128 : (kc + 1) * 128, :])
                cw_sb.append(t)
            wih_sb, whh_sb = [], []
            for kc in range(2):
                t = pool.tile([128, 3 * H], F32, tag=f"wih{kc}")
                nc.sync.dma_start(t[:], wihT[kc * 128 : (kc + 1) * 128, :])
                wih_sb.append(t)
                t = pool.tile([128, 3 * H], F32, tag=f"whh{kc}")
                nc.sync.dma_start(t[:], whhT[kc * 128 : (kc + 1) * 128, :])
                whh_sb.append(t)
            ow_sb = []
            for hc in range(2):
                t = pool.tile([128, VL], BF16, tag=f"ow{hc}")
                nc.sync.dma_start(t[:], owT[hc * 128 : (hc + 1) * 128, :])
                ow_sb.append(t)

            # ---- softmax normalization ----
            # ctx diag extraction: PSUM -> SBUF -> DRAM, then one diagonal
            # DMA read (DRAM APs are flat, so a (BL*H + H)-stride works)
            ctx_all = pool.tile([BL, BL * H], F32, tag="ctxall")
            nc.scalar.copy(ctx_all[:], ctx_ps[:])
            scr = dram.tile([BL, BL * H], F32, tag="scr")
            nc.sync.dma_start(scr[:], ctx_all[:])
            ctx_raw = pool.tile([BL, H], F32, tag="ctxraw")
            diag_ap = bass.AP(scr[:].tensor, 0, [[BL * H + H, BL], [1, H]])
            nc.sync.dma_start(ctx_raw[:], diag_ap)
            psctx.close()

            psum_t = psctx.enter_context(
                tc.tile_pool(name="pt", bufs=2, space="PSUM")
            )
            psum_g = psctx.enter_context(
                tc.tile_pool(name="pg", bufs=4, space="PSUM")
            )
            psum_a = psctx.enter_context(
                tc.tile_pool(name="pa2", bufs=1, space="PSUM")
            )

            def pe_transpose(dst_ap, src_ap, p, f):
                """dst[f, p] = src[p, f] via PE transpose + ACT copy."""
                t_ps = psum_t.tile([128, 128], F32, tag="tps")
                nc.tensor.transpose(t_ps[:f, :p], src_ap, id_sb[:p, :p])
                nc.scalar.copy(dst_ap, t_ps[:f, :p])

            # d_col [16,1], d_row [1,16], then broadcast of 1/d to [128,16]
            sm = psum_a.tile([128, BL], F32, tag="sm")
            nc.tensor.matmul(sm[:BL, :1], rs[:], ones_c[:], start=True, stop=True)
            rec_c = pool.tile([BL, 1], F32, tag="recc")
            nc.vector.reciprocal(rec_c[:], sm[:BL, :1])
            # ctx scaled (stores + attn path deferred past the AllGather)
            ctx_sb = pool.tile([BL, H], F32, tag="ctxsb")
            nc.vector.tensor_scalar_mul(ctx_sb[:], ctx_raw[:], rec_c[:])

            # ---- combine + GRU (all h-major / transposed) ----
            # xT [512, 16] = [embT; ctxT] packed as 4 chunks of 16 cols
            xT = pool.tile([128, 4 * BL], F32, tag="xT")
            for src, base in ((emb_sb, 0), (ctx_sb, 2)):
                for hc in range(2):
                    pe_transpose(
                        xT[:, (base + hc) * BL : (base + hc + 1) * BL],
                        src[:, hc * 128 : (hc + 1) * 128],
                        BL,
                        128,
                    )

            # x2T = relu(combW @ x + comb_b)  [2 chunks of 128 h]
            x2T = pool.tile([128, 2 * BL], F32, tag="x2T")
            for hc in range(2):
                g_ps = psum_g.tile([128, BL], F32, tag="gps")
                for kc in range(4):
                    nc.tensor.matmul(
                        g_ps[:],
                        cw_sb[kc][:, hc * 128 : (hc + 1) * 128],
                        xT[:, kc * BL : (kc + 1) * BL],
                        start=(kc == 0),
                        stop=(kc == 3),
                    )
                nc.scalar.activation(
                    x2T[:, hc * BL : (hc + 1) * BL],
                    g_ps[:],
                    AF.Relu,
                    bias=combB_sb[:, hc : hc + 1],
                )

            # hT [256, 16]
            hT = pool.tile([128, 2 * BL], F32, tag="hT")
            for hc in range(2):
                pe_transpose(
                    hT[:, hc * BL : (hc + 1) * BL],
                    hid_sb[:, hc * 128 : (hc + 1) * 128],
                    BL,
                    128,
                )

            # gates r, z: sigmoid(Wih_g @ x2 + Whh_g @ h + (bih+bhh)_g)
            rz = pool.tile([128, 4 * BL], F32, tag="rz")  # r0 r1 z0 z1
            for gc in range(4):
                g_ps = psum_g.tile([128, BL], F32, tag="gps")
                for kc in range(2):
                    nc.tensor.matmul(
                        g_ps[:],
                        wih_sb[kc][:, gc * 128 : (gc + 1) * 128],
                        x2T[:, kc * BL : (kc + 1) * BL],
                        start=(kc == 0),
                        stop=False,
                    )
                for kc in range(2):
                    nc.tensor.matmul(
                        g_ps[:],
                        whh_sb[kc][:, gc * 128 : (gc + 1) * 128],
                        hT[:, kc * BL : (kc + 1) * BL],
                        start=False,
                        stop=(kc == 1),
                    )
                nc.scalar.activation(
                    rz[:, gc * BL : (gc + 1) * BL],
                    g_ps[:],
                    AF.Sigmoid,
                    bias=brz_sb[:, gc : gc + 1],
                )

            # n = tanh(gx_n + bih_n + r * (gh_n + bhh_n)); h' = n + z*(h-n)
            hnT = pool.tile([128, 2 * BL], F32, tag="hnT")
            for i in range(2):
                gc = 4 + i
                gx_ps = psum_g.tile([128, BL], F32, tag="gps")
                for kc in range(2):
                    nc.tensor.matmul(
                        gx_ps[:],
                        wih_sb[kc][:, gc * 128 : (gc + 1) * 128],
                        x2T[:, kc * BL : (kc + 1) * BL],
                        start=(kc == 0),
                        stop=(kc == 1),
                    )
                gh_ps = psum_g.tile([128, BL], F32, tag="gps")
                for kc in range(2):
                    nc.tensor.matmul(
                        gh_ps[:],
                        whh_sb[kc][:, gc * 128 : (gc + 1) * 128],
                        hT[:, kc * BL : (kc + 1) * BL],
                        start=(kc == 0),
                        stop=(kc == 1),
                    )
                ghn = spool.tile([128, BL], F32, tag="ghn")
                nc.scalar.activation(
                    ghn[:], gh_ps[:], AF.Identity, bias=bhhn_sb[:, i : i + 1]
                )
                t1 = spool.tile([128, BL], F32, tag="t1")
                nc.vector.tensor_mul(t1[:], rz[:, i * BL : (i + 1) * BL], ghn[:])
                t2 = spool.tile([128, BL], F32, tag="t2")
                nc.vector.tensor_add(t2[:], gx_ps[:], t1[:])
                nT = spool.tile([128, BL], F32, tag="nT")
                nc.scalar.activation(nT[:], t2[:], AF.Tanh, bias=bihn_sb[:, i : i + 1])
                d1 = spool.tile([128, BL], F32, tag="d1")
                nc.vector.tensor_sub(d1[:], hT[:, i * BL : (i + 1) * BL], nT[:])
                d2 = spool.tile([128, BL], F32, tag="d2")
                nc.vector.tensor_mul(d2[:], rz[:, (2 + i) * BL : (3 + i) * BL], d1[:])
                nc.vector.tensor_add(hnT[:, i * BL : (i + 1) * BL], nT[:], d2[:])

            # h_new output (b-major) + collective AllGather
            hn_sb = pool.tile([BL, H], F32, tag="hnsb")
            for hc in range(2):
                pe_transpose(
                    hn_sb[:, hc * 128 : (hc + 1) * 128],
                    hnT[:, hc * BL : (hc + 1) * BL],
                    128,
                    BL,
                )
            cc_in = dram.tile([BL, H], F32, tag="ccin")
            cc_out = dram.tile([B, H], F32, tag="ccout")
            nc.sync.dma_start(cc_in[:], hn_sb[:])
            nc.gpsimd.collective_compute(
                "AllGather",
                OP.bypass,
                replica_groups=rg,
                ins=[cc_in.opt()],
                outs=[cc_out.opt()],
            )

            # overlapped with the AllGather: output stores + attn_w path
            nc.sync.dma_start(hnew_o[:], hn_sb[:])
            nc.sync.dma_start(ctx_o[:], ctx_sb[:])
            sm2 = psum_a.tile([128, BL], F32, tag="sm")
            nc.tensor.matmul(sm2[:1, :BL], ones_c[:], rs[:], start=True, stop=True)
            rec_r = pool.tile([1, BL], F32, tag="recr")
            nc.vector.reciprocal(rec_r[:], sm2[:1, :BL])
            sm3 = psum_a.tile([128, BL], F32, tag="sm")
            nc.tensor.matmul(sm3[:, :BL], ones_r[:], rec_r[:], start=True, stop=True)
            rec_all = pool.tile([128, BL], F32, tag="recall")
            nc.scalar.copy(rec_all[:], sm3[:, :BL])
            w_all = pool.tile([128, NT * BL], F32, tag="wall")
            for jj in range(NT):
                nc.vector.tensor_mul(
                    w_all[:, jj * BL : (jj + 1) * BL], p_tiles[jj][:], rec_all[:]
                )
            attn_v = attn_o.rearrange("b (j t) -> j b t", j=NT)
            for half in range(2):
                t_sb = pool.tile([128, 128], F32, tag=f"wt{half}")
                pe_transpose(
                    t_sb[:], w_all[:, half * 128 : (half + 1) * 128], 128, 128
                )
                nc.sync.dma_start(attn_v[half * 8 : (half + 1) * 8], t_sb[:])

            h_all = pool.tile([B, H], F32, tag="hall")
            nc.sync.dma_start(h_all[:], cc_out[:])
            for hc in range(2):
                pe_transpose(
                    hallT[:, hc * B : (hc + 1) * B],
                    h_all[:, hc * 128 : (hc + 1) * 128],
                    128,
                    128,
                )

        # ---- logits (all 128 b x local 4000 v) ----
        with tc.tile_pool(name="plg", bufs=NVC, space="PSUM") as psum_lg:
            lg_ps = []
            for vc in range(NVC):
                ps = psum_lg.tile([B, VC], F32, tag="lgps")
                for hc in range(2):
                    nc.tensor.matmul(
                        ps[:],
                        hallT[:, hc * B : (hc + 1) * B],
                        ow_sb[hc][:, vc * VC : (vc + 1) * VC],
                        start=(hc == 0),
                        stop=False,
                    )
                nc.tensor.matmul(
                    ps[:],
                    onesrb_sb[:],
                    ob_sb[:, vc * VC : (vc + 1) * VC],
                    start=False,
                    stop=True,
                )
                lg_ps.append(ps)
                pexp = prodp.tile([B, VC], F32, tag="pexp")
                nc.scalar.activation(
                    pexp[:], ps[:], AF.Exp, accum_out=se[:, vc : vc + 1]
                )

            se_t = pool.tile([B, 1], F32, tag="set")
            nc.vector.reduce_sum(se_t[:], se[:], axis=AX.X)
            cc2_in = dram.tile([B, 1], F32, tag="cc2in")
            cc2_out = dram.tile([B, 1], F32, tag="cc2out")
            nc.sync.dma_start(cc2_in[:], se_t[:])
            nc.gpsimd.collective_compute(
                "AllReduce",
                OP.add,
                replica_groups=rg,
                ins=[cc2_in.opt()],
                outs=[cc2_out.opt()],
            )
            se_g = pool.tile([B, 1], F32, tag="seg")
            nc.sync.dma_start(se_g[:], cc2_out[:])
            lse = pool.tile([B, 1], F32, tag="lse")
            nc.scalar.activation(lse[:], se_g[:], AF.Ln)
            nc.vector.tensor_scalar_mul(nlse[:], lse[:], -1.0)

            for vc in range(NVC):
                lp = prodp.tile([B, VC], F32, tag="lp")
                nc.scalar.activation(lp[:], lg_ps[vc][:], AF.Identity, bias=nlse[:])
                nc.sync.dma_start(logp_o[:, vc * VC : (vc + 1) * VC], lp[:])

    nc.finalize()
    return nc


def shard_inputs(
    tokens, hidden, encoder_outputs, emb_table, attn_W, attn_b,
    comb_W, comb_b, gru_Wih, gru_Whh, gru_bih, gru_bhh, out_W, out_b,
):
    import ml_dtypes

    f = np.float32
    bf16 = ml_dtypes.bfloat16
    tokens = np.asarray(tokens).astype(np.int64)
    hidden = np.asarray(hidden, f)
    enc = np.asarray(encoder_outputs, f)
    emb_table = np.asarray(emb_table, f)
    web = np.tile(np.asarray(attn_W, f)[:, H:].astype(bf16), (128, BL))
    combWT = np.ascontiguousarray(np.asarray(comb_W, f).T)
    combB = np.ascontiguousarray(np.asarray(comb_b, f).reshape(2, 128).T)
    wihT = np.ascontiguousarray(np.asarray(gru_Wih, f).T)
    whhT = np.ascontiguousarray(np.asarray(gru_Whh, f).T)
    bih = np.asarray(gru_bih, f)
    bhh = np.asarray(gru_bhh, f)
    brz = np.ascontiguousarray((bih + bhh)[: 2 * H].reshape(4, 128).T)
    bihn = np.ascontiguousarray(bih[2 * H :].reshape(2, 128).T)
    bhhn = np.ascontiguousarray(bhh[2 * H :].reshape(2, 128).T)
    out_W = np.asarray(out_W, f)
    out_b = np.asarray(out_b, f)
    shared = dict(
        web=web, combWT=combWT, combB=combB, wihT=wihT, whhT=whhT,
        brz=brz, bihn=bihn, bhhn=bhhn,
        onescol=np.ones((128, 1), f), onesrow=np.ones((1, 128), f),
        onesrb=np.ones((1, 128), bf16),
        ident=np.eye(128, dtype=f),
    )
    in_maps = []
    for c in range(NCORES):
        b0, v0 = c * BL, c * VL
        m = dict(shared)
        m["enc"] = np.ascontiguousarray(
            enc[:, b0 : b0 + BL, :].reshape(T, BL * H)
        )
        m["emb16"] = np.ascontiguousarray(emb_table[tokens[b0 : b0 + BL, 0]])
        m["hid"] = np.ascontiguousarray(hidden[0, b0 : b0 + BL, :])
        m["owT"] = np.ascontiguousarray(out_W[v0 : v0 + VL].T).astype(bf16)
        m["ob"] = np.ascontiguousarray(out_b[None, v0 : v0 + VL]).astype(bf16)
        in_maps.append(m)
    return in_maps


_NC_CACHE = {}


def _get_nc():
    if "nc" not in _NC_CACHE:
        _NC_CACHE["nc"] = build_nc()
    return _NC_CACHE["nc"]


def assemble_outputs(results):
    logp = np.concatenate([r["logp_o"] for r in results], axis=1)[None]
    hnew = np.concatenate([r["hnew_o"] for r in results], axis=0)[None]
    attn = np.concatenate([r["attn_o"] for r in results], axis=0)[..., None]
    ctxo = np.concatenate([r["ctx_o"] for r in results], axis=0)[None]
    return logp, hnew, attn, ctxo


def kernel(**inputs):
    _install_birfix()
    from concourse.bass_utils import run_bass_kernel_spmd

    nc = _get_nc()
    in_maps = shard_inputs(**inputs)
    res = run_bass_kernel_spmd(nc, in_maps, list(range(NCORES)))
    return assemble_outputs(res.results)
